# revision 2
# baseline (speedup 1.0000x reference)
"""InfoNCE loss kernel for Trainium2 (8 NeuronCores, Bass/Tile).

loss = mean_i [ lse_j S[i,j] + lse_j S[j,i] - 2*S[i,i] ],  S = t_hat @ c_hat^T
with t_hat/c_hat the row-l2-normalized text/ctr embeddings [8192, 768].

Host prep (cheap, O(bs*d) numpy): l2-normalize in f32, scale by 64 and
quantize to fp8-e4m3 (so the device matmul computes 4096*S), pack both
operands into the DoubleRow [128, 2, n] two-k-plane layout, and take the
diagonal dot products directly in f64.

Device (data-parallel: core k owns text rows i in [1024k, 1024(k+1)) and
scores them against all 8192 ctr rows j):
  - S^T row-blocks (j on partitions, i free) via fp8 DoubleRow matmuls:
    2 k-planes per instruction at 0.5 PE cycles/column — 4x bf16 rate.
  - exp with the 2^-12 rescale folded into the ScalarE activation scale,
    one [128,1024] instruction per j-block writing bf16 e-tiles (the exp
    chain, 64 x ~1.26us, is the critical path; PSUM triple-buffered so it
    never bubbles).
  - colsum partials (sum over this core's i) on DVE via
    scalar_tensor_tensor accum_out at 2x bf16 rate.
  - rowsum (sum over all j) via ones-vector matmuls accumulated in PSUM,
    deferred a few iterations so PE never waits on the exp chain.

Host gathers: sum partial colsums over cores, take logs, mean — O(bs).
"""

import sys

if "/opt/trn_rl_repo" not in sys.path:
    sys.path.insert(0, "/opt/trn_rl_repo")

import numpy as np

BS = 8192
DIM = 768
NCORES = 8
BLK = BS // NCORES  # 1024 text rows per core
P = 128
DD = DIM // (2 * P)  # 3 double-k-plane contraction chunks
NMP = BS // P  # 64 j-blocks per core
NJC = 16  # ct2 DMA j-chunks (first chunk gates the first matmul)
DEFER = 2  # rowsum matmul deferral depth (e-tiles)
QSCALE = 64.0  # fp8 pre-scale; the matmul yields 4096*S
EXPSCALE = 1.0 / 4096.0

_CACHE = {}


def _build_bass():
    import concourse.bass as bass
    import concourse.mybir as mybir
    from concourse.tile import TileContext
    from contextlib import ExitStack

    f32 = mybir.dt.float32
    bf16 = mybir.dt.bfloat16
    fp8 = mybir.dt.float8e4
    ADD = mybir.AluOpType.add
    MULT = mybir.AluOpType.mult
    AF = mybir.ActivationFunctionType
    DR = mybir.MatmulPerfMode.DoubleRow

    nc = bass.Bass()

    ct2 = nc.dram_tensor("ct2", [DD * P, 2, BS], fp8, kind="ExternalInput")
    tt2 = nc.dram_tensor("tt2", [DD * P, 2, BLK], fp8, kind="ExternalInput")
    colsum_out = nc.dram_tensor("colsum_out", [P, NMP], f32, kind="ExternalOutput")
    rowsum_out = nc.dram_tensor("rowsum_out", [1, BLK], f32, kind="ExternalOutput")

    with TileContext(nc) as tc, ExitStack() as ctx:
        consts = ctx.enter_context(tc.tile_pool(name="consts", bufs=1))
        persist = ctx.enter_context(tc.tile_pool(name="persist", bufs=1))
        epool = ctx.enter_context(tc.tile_pool(name="epool", bufs=4))
        scpool = ctx.enter_context(tc.tile_pool(name="scpool", bufs=2))
        spsum = ctx.enter_context(tc.tile_pool(name="spsum", bufs=3, space="PSUM"))
        rpsum = ctx.enter_context(tc.tile_pool(name="rpsum", bufs=1, space="PSUM"))

        ones = consts.tile([P, 1], bf16, tag="ones")
        nc.vector.memset(ones, 1.0)
        zeros = consts.tile([P, BLK], bf16, tag="zeros")
        nc.vector.memset(zeros, 0.0)

        ct2_sb = [
            persist.tile([P, 2, BS], fp8, tag=f"ct2_{d}", name=f"ct2_{d}")
            for d in range(DD)
        ]
        tt2_sb = [
            persist.tile([P, 2, BLK], fp8, tag=f"tt2_{d}", name=f"tt2_{d}")
            for d in range(DD)
        ]
        colsum_cols = persist.tile([P, NMP], f32, tag="colsum_cols", name="colsum_cols")
        rowsum_sb = persist.tile([1, BLK], f32, tag="rowsum_sb", name="rowsum_sb")

        # The first S-group needs all of tt2 plus ct2 j-chunk 0.  tt2 goes
        # out on the gpsimd/scalar DGE queues so it doesn't serialize behind
        # ct2 on the SP queue; ct2 streams in j-chunks so mp 0 starts early.
        tt2_queues = [nc.gpsimd, nc.scalar, nc.gpsimd]
        for d in range(DD):
            tt2_queues[d].dma_start(
                out=tt2_sb[d], in_=tt2[d * P : (d + 1) * P, :, :]
            )
        jcw = BS // NJC
        for jc in range(NJC):
            for d in range(DD):
                nc.sync.dma_start(
                    out=ct2_sb[d][:, :, jc * jcw : (jc + 1) * jcw],
                    in_=ct2[d * P : (d + 1) * P, :, jc * jcw : (jc + 1) * jcw],
                )

        rowsum_ps = [
            rpsum.tile([1, 512], f32, tag=f"rs{i}", name=f"rs{i}") for i in range(2)
        ]

        pending = []  # e-tiles awaiting deferred rowsum matmuls
        chunks_done = [0]
        n_chunks_total = 2 * NMP

        def emit_s_matmuls(mp):
            ps = spsum.tile([P, 1024], f32, tag="sps", name="sps")
            for c in range(2):
                for d in range(DD):
                    nc.tensor.matmul(
                        ps[:, c * 512 : (c + 1) * 512],
                        ct2_sb[d][:, :, mp * P : (mp + 1) * P],
                        tt2_sb[d][:, :, c * 512 : (c + 1) * 512],
                        start=(d == 0),
                        stop=(d == DD - 1),
                        perf_mode=DR,
                    )
            return ps

        def emit_rowsum(e):
            for c in range(2):
                nc.tensor.matmul(
                    rowsum_ps[c],
                    ones,
                    e[:, c * 512 : (c + 1) * 512],
                    start=(chunks_done[0] + c < 2),
                    stop=(chunks_done[0] + c >= n_chunks_total - 2),
                    skip_group_check=True,
                )
            chunks_done[0] += 2

        def emit_exp_and_sums(mp, ps):
            e = epool.tile([P, 1024], bf16, tag="e", name="e")
            nc.scalar.activation(e, ps, AF.Exp, scale=EXPSCALE)
            scr = scpool.tile([P, 1024], bf16, tag="scr", name="scr")
            nc.vector.scalar_tensor_tensor(
                out=scr,
                in0=e,
                scalar=1.0,
                in1=zeros,
                op0=MULT,
                op1=ADD,
                accum_out=colsum_cols[:, mp : mp + 1],
            )
            pending.append(e)

        prev = None
        for mp in range(NMP):
            ps = emit_s_matmuls(mp)
            while len(pending) > DEFER:
                emit_rowsum(pending.pop(0))
            if prev is not None:
                emit_exp_and_sums(*prev)
            prev = (mp, ps)
        emit_exp_and_sums(*prev)
        while pending:
            emit_rowsum(pending.pop(0))

        for c in range(2):
            nc.vector.tensor_copy(
                rowsum_sb[:, c * 512 : (c + 1) * 512], rowsum_ps[c]
            )
        nc.sync.dma_start(out=rowsum_out[:, :], in_=rowsum_sb)
        nc.sync.dma_start(out=colsum_out[:, :], in_=colsum_cols)

    _split_multiwaits(nc, mybir)
    return nc


def _split_multiwaits(nc, mybir):
    """This container's walrus accepts only one sync-wait command per
    instruction; Tile emits several.  Move all-but-one wait onto a NoOp
    inserted just before, on the same engine (in-order sequencers make this
    semantically identical)."""
    for f in nc.m.functions:
        for bb in f.blocks:
            insts = bb.instructions
            out = []
            changed = False
            for inst in insts:
                si = getattr(inst, "sync_info", None)
                ow = list(si.on_wait) if (si is not None and si.on_wait) else []
                if len(ow) > 1:
                    changed = True
                    for wi, w in enumerate(ow[:-1]):
                        out.append(
                            mybir.InstNoOp(
                                name=f"{inst.name}-wsplit{wi}",
                                engine=inst.engine,
                                sync_info=mybir.SyncInfo(on_wait=[w], on_update=[]),
                            )
                        )
                    inst.sync_info = mybir.SyncInfo(
                        on_wait=ow[-1:], on_update=list(si.on_update or [])
                    )
                out.append(inst)
            if changed:
                bb.instructions = out


def _get_nc():
    if "nc" not in _CACHE:
        _CACHE["nc"] = _build_bass()
    return _CACHE["nc"]


def _pack_double_row(mat_t):
    """[DIM, n] -> [DD*P, 2, n] fp8 DoubleRow two-k-plane layout."""
    d, n = mat_t.shape
    return np.ascontiguousarray(
        mat_t.reshape(DD, 2, P, n).transpose(0, 2, 1, 3).reshape(DD * P, 2, n)
    )


def _run(in_maps, trace=False):
    from concourse.bass_utils import run_bass_kernel_spmd

    nc = _get_nc()
    try:
        return run_bass_kernel_spmd(
            nc, in_maps, core_ids=list(range(NCORES)), trace=trace
        )
    except ModuleNotFoundError:
        # NTFF profile hook unavailable in this container; rerun untraced.
        return run_bass_kernel_spmd(
            nc, in_maps, core_ids=list(range(NCORES)), trace=False
        )


def kernel(text_emb, ctr_emb, _trace=False, _want_result_obj=False):
    import ml_dtypes

    t32 = np.asarray(text_emb, dtype=np.float32)
    c32 = np.asarray(ctr_emb, dtype=np.float32)
    tn = t32 / np.maximum(np.linalg.norm(t32, axis=1, keepdims=True), 1e-8)
    cn = c32 / np.maximum(np.linalg.norm(c32, axis=1, keepdims=True), 1e-8)
    diag_total = float(
        np.einsum("ij,ij->", tn.astype(np.float64), cn.astype(np.float64))
    )

    fp8 = ml_dtypes.float8_e4m3fn
    qt = (tn * QSCALE).astype(fp8)
    qc = (cn * QSCALE).astype(fp8)

    ct2 = _pack_double_row(qc.T)
    in_maps = [
        {
            "ct2": ct2,
            "tt2": _pack_double_row(
                np.ascontiguousarray(qt[k * BLK : (k + 1) * BLK].T)
            ),
        }
        for k in range(NCORES)
    ]
    res = _run(in_maps, trace=_trace)

    colsum_total = np.zeros(BS, dtype=np.float64)
    rowsum_all = np.empty(BS, dtype=np.float64)
    for k, r in enumerate(res.results):
        # colsum_out[p, mp] = partial colsum for j = mp*128 + p
        colsum_total += r["colsum_out"].astype(np.float64).T.reshape(-1)
        rowsum_all[k * BLK : (k + 1) * BLK] = r["rowsum_out"][0]

    total = (
        np.log(rowsum_all).sum() + np.log(colsum_total).sum() - 2.0 * diag_total
    )
    out = np.float32(total / BS)
    if _want_result_obj:
        return out, res
    return out


# revision 6
# speedup vs baseline: 1.0011x; 1.0011x over previous
"""InfoNCE loss kernel for Trainium2 (8 NeuronCores, Bass/Tile).

loss = mean_i [ lse_j S[i,j] + lse_j S[j,i] - 2*S[i,i] ],  S = t_hat @ c_hat^T
with t_hat/c_hat the row-l2-normalized text/ctr embeddings [8192, 768].

Host prep (cheap, O(bs*d) numpy): l2-normalize in f32, scale by 64 and
quantize to fp8-e4m3 (so the device matmul computes 4096*S), pack both
operands into the DoubleRow [128, 2, n] two-k-plane layout, and take the
diagonal dot products directly in f64.

Device (data-parallel: core k owns text rows i in [1024k, 1024(k+1)) and
scores them against all 8192 ctr rows j):
  - S^T row-blocks (j on partitions, i free) via fp8 DoubleRow matmuls:
    2 k-planes per instruction at 0.5 PE cycles/column — 4x bf16 rate.
  - exp with the 2^-12 rescale folded into the ScalarE activation scale,
    one [128,1024] instruction per j-block writing bf16 e-tiles (the exp
    chain, 64 x ~1.26us, is the critical path; PSUM triple-buffered so it
    never bubbles).
  - colsum partials (sum over this core's i) on DVE via
    scalar_tensor_tensor accum_out at 2x bf16 rate.
  - rowsum (sum over all j) via ones-vector matmuls accumulated in PSUM,
    deferred a few iterations so PE never waits on the exp chain.

Host gathers: sum partial colsums over cores, take logs, mean — O(bs).
"""

import sys

if "/opt/trn_rl_repo" not in sys.path:
    sys.path.insert(0, "/opt/trn_rl_repo")

import numpy as np

BS = 8192
DIM = 768
NCORES = 8
BLK = BS // NCORES  # 1024 text rows per core
P = 128
DD = DIM // (2 * P)  # 3 double-k-plane contraction chunks
NMP = BS // P  # 64 j-blocks per core
NJC = 16  # ct2 DMA j-chunks (first chunk gates the first matmul)
DEFER = 2  # rowsum matmul deferral depth (e-tiles)
QSCALE = 64.0  # fp8 pre-scale; the matmul yields 4096*S
EXPSCALE = 1.0 / 4096.0

_CACHE = {}


def _build_bass():
    import concourse.bass as bass
    import concourse.mybir as mybir
    from concourse.tile import TileContext
    from contextlib import ExitStack

    f32 = mybir.dt.float32
    bf16 = mybir.dt.bfloat16
    fp8 = mybir.dt.float8e4
    ADD = mybir.AluOpType.add
    MULT = mybir.AluOpType.mult
    AF = mybir.ActivationFunctionType
    DR = mybir.MatmulPerfMode.DoubleRow

    nc = bass.Bass()

    ct2 = nc.dram_tensor("ct2", [DD * P, 2, BS], fp8, kind="ExternalInput")
    tt2 = nc.dram_tensor("tt2", [DD * P, 2, BLK], fp8, kind="ExternalInput")
    colsum_out = nc.dram_tensor("colsum_out", [P, NMP], f32, kind="ExternalOutput")
    rowsum_out = nc.dram_tensor("rowsum_out", [1, BLK], f32, kind="ExternalOutput")

    with TileContext(nc) as tc, ExitStack() as ctx:
        consts = ctx.enter_context(tc.tile_pool(name="consts", bufs=1))
        persist = ctx.enter_context(tc.tile_pool(name="persist", bufs=1))
        epool = ctx.enter_context(tc.tile_pool(name="epool", bufs=4))
        scpool = ctx.enter_context(tc.tile_pool(name="scpool", bufs=2))
        spsum = ctx.enter_context(tc.tile_pool(name="spsum", bufs=3, space="PSUM"))
        rpsum = ctx.enter_context(tc.tile_pool(name="rpsum", bufs=1, space="PSUM"))

        ones = consts.tile([P, 1], bf16, tag="ones")
        nc.vector.memset(ones, 1.0)
        zeros = consts.tile([P, BLK], bf16, tag="zeros")
        nc.vector.memset(zeros, 0.0)

        ct2_sb = [
            persist.tile([P, 2, BS], fp8, tag=f"ct2_{d}", name=f"ct2_{d}")
            for d in range(DD)
        ]
        tt2_sb = [
            persist.tile([P, 2, BLK], fp8, tag=f"tt2_{d}", name=f"tt2_{d}")
            for d in range(DD)
        ]
        colsum_cols = persist.tile([P, NMP], f32, tag="colsum_cols", name="colsum_cols")
        rowsum_sb = persist.tile([1, BLK], f32, tag="rowsum_sb", name="rowsum_sb")

        # The first S-group needs all of tt2 plus ct2 j-chunk 0.  tt2 goes
        # out on the gpsimd/scalar DGE queues so it doesn't serialize behind
        # ct2 on the SP queue; ct2 streams in j-chunks so mp 0 starts early.
        tt2_queues = [nc.gpsimd, nc.scalar, nc.gpsimd]
        for d in range(DD):
            tt2_queues[d].dma_start(
                out=tt2_sb[d], in_=tt2[d * P : (d + 1) * P, :, :]
            )
        plan = [(0, P), (P, 512)] + [
            (512 * i, 512 * (i + 1)) for i in range(1, NJC)
        ]
        for a, b in plan:
            for d in range(DD):
                nc.sync.dma_start(
                    out=ct2_sb[d][:, :, a:b],
                    in_=ct2[d * P : (d + 1) * P, :, a:b],
                )

        # Warm up the ScalarE exp spline table during the DMA prologue so the
        # first real activation doesn't pay the ~2.7us ACT_TABLE_LOAD on
        # hardware.  Issued after the tt2 dispatch that shares the ACT
        # sequencer, so the table load overlaps the DMA transfers.
        warm = consts.tile([P, 1], f32, tag="warm")
        nc.scalar.activation(warm, ones, AF.Exp, scale=1.0)

        rowsum_ps = [
            rpsum.tile([1, 512], f32, tag=f"rs{i}", name=f"rs{i}") for i in range(2)
        ]

        pending = []  # e-tiles awaiting deferred rowsum matmuls
        chunks_done = [0]
        n_chunks_total = 2 * NMP

        def emit_s_matmuls(mp):
            ps = spsum.tile([P, 1024], f32, tag="sps", name="sps")
            for c in range(2):
                for d in range(DD):
                    nc.tensor.matmul(
                        ps[:, c * 512 : (c + 1) * 512],
                        ct2_sb[d][:, :, mp * P : (mp + 1) * P],
                        tt2_sb[d][:, :, c * 512 : (c + 1) * 512],
                        start=(d == 0),
                        stop=(d == DD - 1),
                        perf_mode=DR,
                    )
            return ps

        def emit_rowsum(e):
            for c in range(2):
                nc.tensor.matmul(
                    rowsum_ps[c],
                    ones,
                    e[:, c * 512 : (c + 1) * 512],
                    start=(chunks_done[0] + c < 2),
                    stop=(chunks_done[0] + c >= n_chunks_total - 2),
                    skip_group_check=True,
                )
            chunks_done[0] += 2

        def emit_exp_and_sums(mp, ps):
            e = epool.tile([P, 1024], bf16, tag="e", name="e")
            nc.scalar.activation(e, ps, AF.Exp, scale=EXPSCALE)
            scr = scpool.tile([P, 1024], bf16, tag="scr", name="scr")
            nc.vector.scalar_tensor_tensor(
                out=scr,
                in0=e,
                scalar=1.0,
                in1=zeros,
                op0=MULT,
                op1=ADD,
                accum_out=colsum_cols[:, mp : mp + 1],
            )
            pending.append(e)

        prev = None
        for mp in range(NMP):
            ps = emit_s_matmuls(mp)
            while len(pending) > DEFER:
                emit_rowsum(pending.pop(0))
            if prev is not None:
                emit_exp_and_sums(*prev)
            prev = (mp, ps)
        emit_exp_and_sums(*prev)
        while pending:
            emit_rowsum(pending.pop(0))

        for c in range(2):
            nc.vector.tensor_copy(
                rowsum_sb[:, c * 512 : (c + 1) * 512], rowsum_ps[c]
            )
        nc.sync.dma_start(out=rowsum_out[:, :], in_=rowsum_sb)
        nc.sync.dma_start(out=colsum_out[:, :], in_=colsum_cols)

    _split_multiwaits(nc, mybir)
    return nc


def _split_multiwaits(nc, mybir):
    """This container's walrus accepts only one sync-wait command per
    instruction; Tile emits several.  Move all-but-one wait onto a NoOp
    inserted just before, on the same engine (in-order sequencers make this
    semantically identical)."""
    for f in nc.m.functions:
        for bb in f.blocks:
            insts = bb.instructions
            out = []
            changed = False
            for inst in insts:
                si = getattr(inst, "sync_info", None)
                ow = list(si.on_wait) if (si is not None and si.on_wait) else []
                if len(ow) > 1:
                    changed = True
                    for wi, w in enumerate(ow[:-1]):
                        out.append(
                            mybir.InstNoOp(
                                name=f"{inst.name}-wsplit{wi}",
                                engine=inst.engine,
                                sync_info=mybir.SyncInfo(on_wait=[w], on_update=[]),
                            )
                        )
                    inst.sync_info = mybir.SyncInfo(
                        on_wait=ow[-1:], on_update=list(si.on_update or [])
                    )
                out.append(inst)
            if changed:
                bb.instructions = out


def _get_nc():
    if "nc" not in _CACHE:
        _CACHE["nc"] = _build_bass()
    return _CACHE["nc"]


def _pack_double_row(mat_t):
    """[DIM, n] -> [DD*P, 2, n] fp8 DoubleRow two-k-plane layout."""
    d, n = mat_t.shape
    return np.ascontiguousarray(
        mat_t.reshape(DD, 2, P, n).transpose(0, 2, 1, 3).reshape(DD * P, 2, n)
    )


def _run(in_maps, trace=False):
    from concourse.bass_utils import run_bass_kernel_spmd

    nc = _get_nc()
    try:
        return run_bass_kernel_spmd(
            nc, in_maps, core_ids=list(range(NCORES)), trace=trace
        )
    except ModuleNotFoundError:
        # NTFF profile hook unavailable in this container; rerun untraced.
        return run_bass_kernel_spmd(
            nc, in_maps, core_ids=list(range(NCORES)), trace=False
        )


def kernel(text_emb, ctr_emb, _trace=False, _want_result_obj=False):
    import ml_dtypes

    t32 = np.asarray(text_emb, dtype=np.float32)
    c32 = np.asarray(ctr_emb, dtype=np.float32)
    tn = t32 / np.maximum(np.linalg.norm(t32, axis=1, keepdims=True), 1e-8)
    cn = c32 / np.maximum(np.linalg.norm(c32, axis=1, keepdims=True), 1e-8)
    diag_total = float(
        np.einsum("ij,ij->", tn.astype(np.float64), cn.astype(np.float64))
    )

    fp8 = ml_dtypes.float8_e4m3fn
    qt = (tn * QSCALE).astype(fp8)
    qc = (cn * QSCALE).astype(fp8)

    ct2 = _pack_double_row(qc.T)
    in_maps = [
        {
            "ct2": ct2,
            "tt2": _pack_double_row(
                np.ascontiguousarray(qt[k * BLK : (k + 1) * BLK].T)
            ),
        }
        for k in range(NCORES)
    ]
    res = _run(in_maps, trace=_trace)

    colsum_total = np.zeros(BS, dtype=np.float64)
    rowsum_all = np.empty(BS, dtype=np.float64)
    for k, r in enumerate(res.results):
        # colsum_out[p, mp] = partial colsum for j = mp*128 + p
        colsum_total += r["colsum_out"].astype(np.float64).T.reshape(-1)
        rowsum_all[k * BLK : (k + 1) * BLK] = r["rowsum_out"][0]

    total = (
        np.log(rowsum_all).sum() + np.log(colsum_total).sum() - 2.0 * diag_total
    )
    out = np.float32(total / BS)
    if _want_result_obj:
        return out, res
    return out
